# revision 3
# baseline (speedup 1.0000x reference)
"""Ctx-MHAtt Trainium2 kernel (Bass/Tile), batch-parallel over 8 NeuronCores.

Per core (one batch element):
  ghT = (Wq*a) @ qT + (a*bq) [+ beta-scaled caption upsample]   [d,h,n] layout
  khT = Wk @ kT + bk                                            [d,h,n]
  vh  = vT.T @ WvT (+bv)                                        [n,h*d]
  scores(h,qtile) = ghT_h.T @ khT_h  (PSUM), LayerNorm over keys via
  bn_stats, softmax via ACT Exp(scale=rstd, bias=-mu*rstd) + accum row-sum,
  normalize, PE-transpose to attT, out_hT = vh_h.T @ attT, final Y = outT.T@WmT.
All matmuls in float32r (full PE rate at moving>=256, ~1.5e-4 rel err).
Scale factors 1/((1+t)sqrt(d)) and t/((1+t)sqrt(d)) folded into Wq/bq/Wup
on the host (t = tanh(tau))."""

import math

import numpy as np

B, NQ, NK, HD = 8, 1024, 1024, 1024
H, HS, KC = 8, 128, 32
EPS = 1e-5

_CACHE = {}


def _build(flags):
    import concourse.bacc as bacc
    import concourse.tile as tile
    import concourse.mybir as mybir

    F32 = mybir.dt.float32
    F32R = mybir.dt.float32r
    AF = mybir.ActivationFunctionType
    ALU = mybir.AluOpType

    has_bv, has_bc, has_bm, has_ln = (
        flags["bv"], flags["bc"], flags["bm"], flags["ln"])

    nc = bacc.Bacc("TRN2", target_bir_lowering=False, debug=False,
                   num_devices=8)

    def din(name, shape, dt=F32R):
        return nc.dram_tensor(name, shape, dt, kind="ExternalInput").ap()

    qT = din("qT", [HD, NQ])
    kT = din("kT", [HD, NK])
    vT = din("vT", [HD, NK])
    cT = din("cT", [HD, KC])
    WqT = din("WqT", [HD, HD])
    WkT = din("WkT", [HD, HD])
    WvT = din("WvT", [HD, HD])
    WcT = din("WcT", [HD, HD])
    WmT = din("WmT", [HD, HD])
    Wup = din("WupT", [KC, NQ])
    bqs = din("bqs", [128, 8], F32)
    bks = din("bks", [128, 8], F32)
    ident = din("ident", [128, 128], F32R)
    if has_bv:
        bvD = din("bv_r", [1, HD], F32)
    if has_bc:
        bcD = din("bc_r", [1, HD], F32)
    if has_bm:
        bmD = din("bm_r", [1, HD], F32)
    if has_ln:
        lnwD = din("lnw_r", [1, NK], F32)
        lnbD = din("lnb_r", [1, NK], F32)
    Y = nc.dram_tensor("Y", [NQ, HD], F32, kind="ExternalOutput").ap()

    def bcast(ap, p=128):
        import concourse.bass as bass
        return bass.AP(tensor=ap.tensor, offset=ap.offset,
                       ap=[[0, p]] + list(ap.ap)[1:])

    with tile.TileContext(nc) as tc:
        with tc.tile_pool(name="perm", bufs=1) as perm, \
             tc.tile_pool(name="smq", bufs=8) as smq:
            ghT = perm.tile([128, H, NQ], F32R, tag="ghT")
            khT = perm.tile([128, H, NK], F32R, tag="khT")
            vh = perm.tile([128, H, HD], F32R, tag="vh")
            ch = perm.tile([KC, HD], F32R, tag="ch")
            wup = perm.tile([KC, NQ], F32R, tag="wup")
            idt = perm.tile([128, 128], F32R, tag="idt")
            bq_t = perm.tile([128, 8], F32, tag="bq")
            bk_t = perm.tile([128, 8], F32, tag="bk")
            eps_t = perm.tile([128, 1], F32, tag="eps")
            ct = perm.tile([128, 8, KC], F32R, tag="ct")

            nc.vector.memset(eps_t, EPS)
            nc.sync.dma_start(out=wup, in_=Wup)
            nc.sync.dma_start(out=idt, in_=ident)
            nc.sync.dma_start(out=bq_t, in_=bqs)
            nc.sync.dma_start(out=bk_t, in_=bks)
            for i in range(8):
                nc.sync.dma_start(out=ct[:, i, :], in_=cT[i * 128:(i + 1) * 128, :])
            if has_bv:
                bv_t = perm.tile([128, HD], F32, tag="bv")
                nc.sync.dma_start(out=bv_t, in_=bcast(bvD))
            if has_bc:
                bc_t = perm.tile([KC, HD], F32, tag="bc")
                nc.sync.dma_start(out=bc_t, in_=bcast(bcD, KC))
            if has_bm:
                bm_t = perm.tile([128, HD], F32, tag="bm")
                nc.sync.dma_start(out=bm_t, in_=bcast(bmD))
            if has_ln:
                lnw_t = perm.tile([128, NK], F32, tag="lnw")
                lnb_t = perm.tile([128, NK], F32, tag="lnb")
                nc.sync.dma_start(out=lnw_t, in_=bcast(lnwD))
                nc.sync.dma_start(out=lnb_t, in_=bcast(lnbD))

            # ---------------- P0-P3: projections ----------------
            with tc.tile_pool(name="w", bufs=1) as wpool, \
                 tc.tile_pool(name="win", bufs=2) as win, \
                 tc.tile_pool(name="wc", bufs=3) as wcp, \
                 tc.tile_pool(name="psC", bufs=2, space="PSUM") as psC, \
                 tc.tile_pool(name="psA", bufs=4, space="PSUM") as psA:

                # caption projection: ch[c, o] = sum_i cT[i,c] * WcT[i,o]
                pc0 = psC.tile([KC, 512], F32, tag="psc")
                pc1 = psC.tile([KC, 512], F32, tag="psc")
                for i in range(8):
                    wci = wcp.tile([128, HD], F32R, tag="wc")
                    nc.sync.dma_start(out=wci, in_=WcT[i * 128:(i + 1) * 128, :])
                    nc.tensor.matmul(pc0, ct[:, i, :], wci[:, 0:512],
                                     start=(i == 0), stop=(i == 7))
                    nc.tensor.matmul(pc1, ct[:, i, :], wci[:, 512:1024],
                                     start=(i == 0), stop=(i == 7))
                if has_bc:
                    nc.vector.scalar_tensor_tensor(
                        ch[:, 0:512], pc0, 0.0, bc_t[:, 0:512],
                        op0=ALU.add, op1=ALU.add)
                    nc.vector.scalar_tensor_tensor(
                        ch[:, 512:1024], pc1, 0.0, bc_t[:, 512:1024],
                        op0=ALU.add, op1=ALU.add)
                else:
                    nc.vector.tensor_copy(ch[:, 0:512], pc0)
                    nc.vector.tensor_copy(ch[:, 512:1024], pc1)

                # q/k projections into [d, h, n] layouts (+ caption upsample on q)
                for (W_d, x_d, dst, bias_t, add_cu) in (
                        (WqT, qT, ghT, bq_t, True), (WkT, kT, khT, bk_t, False)):
                    wq = wpool.tile([128, 8, HD], F32R, tag="w")
                    for i in range(8):
                        nc.sync.dma_start(out=wq[:, i, :],
                                          in_=W_d[i * 128:(i + 1) * 128, :])
                    for cnk in range(2):
                        n0 = cnk * 512
                        qc = win.tile([128, 8, 512], F32R, tag="in")
                        for i in range(8):
                            nc.sync.dma_start(
                                out=qc[:, i, :],
                                in_=x_d[i * 128:(i + 1) * 128, n0:n0 + 512])
                        for h in range(H):
                            ps = psA.tile([128, 512], F32, tag="psp")
                            for i in range(8):
                                nc.tensor.matmul(
                                    ps, wq[:, i, h * 128:(h + 1) * 128],
                                    qc[:, i, :], start=(i == 0),
                                    stop=(not add_cu and i == 7))
                            if add_cu:
                                nc.tensor.matmul(
                                    ps, ch[:, h * 128:(h + 1) * 128],
                                    wup[:, n0:n0 + 512], start=False, stop=True)
                            nc.vector.tensor_scalar_add(
                                dst[:, h, n0:n0 + 512], ps, bias_t[:, h:h + 1])

                # v projection into [n, o] layout
                wv = wpool.tile([128, 8, HD], F32R, tag="w")
                for i in range(8):
                    nc.sync.dma_start(out=wv[:, i, :],
                                      in_=WvT[i * 128:(i + 1) * 128, :])
                for cnk in range(2):
                    n0 = cnk * 512
                    vc = win.tile([128, 8, 512], F32R, tag="in")
                    for i in range(8):
                        nc.sync.dma_start(
                            out=vc[:, i, :],
                            in_=vT[i * 128:(i + 1) * 128, n0:n0 + 512])
                    for ntl in range(4):
                        nt = cnk * 4 + ntl
                        psa = psA.tile([128, 512], F32, tag="psp")
                        psb = psA.tile([128, 512], F32, tag="psp")
                        for i in range(8):
                            st_ = vc[:, i, ntl * 128:(ntl + 1) * 128]
                            nc.tensor.matmul(psa, st_, wv[:, i, 0:512],
                                             start=(i == 0), stop=(i == 7))
                            nc.tensor.matmul(psb, st_, wv[:, i, 512:1024],
                                             start=(i == 0), stop=(i == 7))
                        if has_bv:
                            nc.vector.scalar_tensor_tensor(
                                vh[:, nt, 0:512], psa, 0.0, bv_t[:, 0:512],
                                op0=ALU.add, op1=ALU.add)
                            nc.vector.scalar_tensor_tensor(
                                vh[:, nt, 512:1024], psb, 0.0, bv_t[:, 512:1024],
                                op0=ALU.add, op1=ALU.add)
                        else:
                            nc.vector.tensor_copy(vh[:, nt, 0:512], psa)
                            nc.vector.tensor_copy(vh[:, nt, 512:1024], psb)

            # ---------------- P4: attention per head ----------------
            with tc.tile_pool(name="outp", bufs=1) as outp:
                outT = outp.tile([128, H, NQ], F32R, tag="outT")
                with tc.tile_pool(name="p4", bufs=2) as p4, \
                     tc.tile_pool(name="attp", bufs=3) as attp, \
                     tc.tile_pool(name="psS", bufs=2, space="PSUM") as psS, \
                     tc.tile_pool(name="psT", bufs=2, space="PSUM") as psT, \
                     tc.tile_pool(name="psV", bufs=2, space="PSUM") as psV:
                    for h in range(H):
                        for g in range(2):
                            attT = p4.tile([128, 8, 512], F32R, tag="attT")
                            for qtl in range(4):
                                qt = g * 4 + qtl
                                ps_s = psS.tile([128, 1024], F32, tag="s")
                                lh = ghT[:, h, qt * 128:(qt + 1) * 128]
                                nc.tensor.matmul(ps_s[:, 0:512], lh,
                                                 khT[:, h, 0:512],
                                                 start=True, stop=True)
                                nc.tensor.matmul(ps_s[:, 512:1024], lh,
                                                 khT[:, h, 512:1024],
                                                 start=True, stop=True)
                                stt = smq.tile([128, 2, 6], F32, tag="st")
                                nc.vector.bn_stats(out=stt[:, 0, :], in_=ps_s[:, 0:512])
                                nc.vector.bn_stats(out=stt[:, 1, :], in_=ps_s[:, 512:1024])
                                mv = smq.tile([128, 2], F32, tag="mv")
                                nc.vector.bn_aggr(out=mv, in_=stt)
                                sq = smq.tile([128, 1], F32, tag="sq")
                                nc.scalar.activation(out=sq, in_=mv[:, 1:2],
                                                     func=AF.Sqrt, bias=eps_t,
                                                     scale=1.0)
                                rv = smq.tile([128, 1], F32, tag="rv")
                                nc.vector.reciprocal(rv, sq)
                                att = attp.tile([128, NK], F32R, tag="att")
                                ssum = smq.tile([128, 1], F32, tag="ss")
                                if has_ln:
                                    tmp = attp.tile([128, NK], F32, tag="lntmp")
                                    nc.vector.scalar_tensor_tensor(
                                        tmp, ps_s, mv[:, 0:1], lnw_t,
                                        op0=ALU.subtract, op1=ALU.mult)
                                    nc.vector.scalar_tensor_tensor(
                                        tmp, tmp, rv, lnb_t,
                                        op0=ALU.mult, op1=ALU.add)
                                    mx = smq.tile([128, 1], F32, tag="mx")
                                    nc.vector.reduce_max(
                                        mx, tmp, axis=mybir.AxisListType.X)
                                    mneg = smq.tile([128, 1], F32, tag="mn")
                                    nc.vector.tensor_scalar_mul(mneg, mx, -1.0)
                                    nc.scalar.activation(
                                        out=att, in_=tmp, func=AF.Exp,
                                        bias=mneg, scale=1.0, accum_out=ssum)
                                else:
                                    be = smq.tile([128, 1], F32, tag="be")
                                    nc.vector.scalar_tensor_tensor(
                                        be, mv[:, 0:1], -1.0, rv,
                                        op0=ALU.mult, op1=ALU.mult)
                                    nc.scalar.activation(
                                        out=att, in_=ps_s, func=AF.Exp,
                                        bias=be, scale=rv, accum_out=ssum)
                                rec = smq.tile([128, 1], F32, tag="rec")
                                nc.vector.reciprocal(rec, ssum)
                                nc.vector.tensor_scalar_mul(att, att, rec)
                                for g4 in range(2):
                                    ps_t = psT.tile([128, 4, 128], F32R, tag="t")
                                    for j in range(4):
                                        kt = g4 * 4 + j
                                        nc.tensor.transpose(
                                            ps_t[:, j, :],
                                            att[:, kt * 128:(kt + 1) * 128], idt)
                                    nc.vector.tensor_copy(
                                        attT[:, g4 * 4:(g4 + 1) * 4,
                                             qtl * 128:(qtl + 1) * 128], ps_t)
                            ps_o = psV.tile([128, 512], F32, tag="o")
                            for kt in range(8):
                                nc.tensor.matmul(
                                    ps_o, vh[:, kt, h * 128:(h + 1) * 128],
                                    attT[:, kt, :],
                                    start=(kt == 0), stop=(kt == 7))
                            nc.vector.tensor_copy(
                                outT[:, h, g * 512:(g + 1) * 512], ps_o)

                # ---------------- P5: output projection ----------------
                with tc.tile_pool(name="wm", bufs=1) as wmp, \
                     tc.tile_pool(name="yp", bufs=2) as yp, \
                     tc.tile_pool(name="psY", bufs=4, space="PSUM") as psY:
                    wm = []
                    for j in range(8):
                        t = wmp.tile([128, HD], F32R, tag=f"wm{j}")
                        nc.sync.dma_start(out=t, in_=WmT[j * 128:(j + 1) * 128, :])
                        wm.append(t)
                    for nt in range(8):
                        py0 = psY.tile([128, 512], F32, tag="y")
                        py1 = psY.tile([128, 512], F32, tag="y")
                        for j in range(8):
                            lh = outT[:, j, nt * 128:(nt + 1) * 128]
                            nc.tensor.matmul(py0, lh, wm[j][:, 0:512],
                                             start=(j == 0), stop=(j == 7))
                            nc.tensor.matmul(py1, lh, wm[j][:, 512:1024],
                                             start=(j == 0), stop=(j == 7))
                        y = yp.tile([128, HD], F32, tag="y")
                        if has_bm:
                            nc.vector.scalar_tensor_tensor(
                                y[:, 0:512], py0, 0.0, bm_t[:, 0:512],
                                op0=ALU.add, op1=ALU.add)
                            nc.vector.scalar_tensor_tensor(
                                y[:, 512:1024], py1, 0.0, bm_t[:, 512:1024],
                                op0=ALU.add, op1=ALU.add)
                        else:
                            nc.vector.tensor_copy(y[:, 0:512], py0)
                            nc.vector.tensor_copy(y[:, 512:1024], py1)
                        nc.sync.dma_start(out=Y[nt * 128:(nt + 1) * 128, :], in_=y)
    nc.compile()
    return nc


def _prep(v, k, q, c, Wv, bv, Wk, bk, Wq, bq, Wc, bc, Wm, bm, Wup, ln_w,
          ln_b, tau):
    t = float(np.tanh(np.float32(tau)))
    alpha = 1.0 / ((1.0 + t) * math.sqrt(HS))
    beta = t * alpha
    C = np.ascontiguousarray
    f32 = np.float32
    flags = dict(
        bv=bool(np.any(bv)), bc=bool(np.any(bc)), bm=bool(np.any(bm)),
        ln=not (np.all(ln_w == 1.0) and np.all(ln_b == 0.0)))
    shared = {
        "WqT": C((np.asarray(Wq, f32) * alpha).T),
        "WkT": C(np.asarray(Wk, f32).T),
        "WvT": C(np.asarray(Wv, f32).T),
        "WcT": C(np.asarray(Wc, f32).T),
        "WmT": C(np.asarray(Wm, f32).T),
        "WupT": C(np.asarray(Wup, f32).T * beta),
        "bqs": C((np.asarray(bq, f32) * alpha).reshape(8, 128).T),
        "bks": C(np.asarray(bk, f32).reshape(8, 128).T),
        "ident": np.eye(128, dtype=f32),
    }
    if flags["bv"]:
        shared["bv_r"] = np.asarray(bv, f32).reshape(1, HD)
    if flags["bc"]:
        shared["bc_r"] = np.asarray(bc, f32).reshape(1, HD)
    if flags["bm"]:
        shared["bm_r"] = np.asarray(bm, f32).reshape(1, HD)
    if flags["ln"]:
        shared["lnw_r"] = np.asarray(ln_w, f32).reshape(1, NK)
        shared["lnb_r"] = np.asarray(ln_b, f32).reshape(1, NK)
    in_maps = []
    for b in range(B):
        m = dict(shared)
        m["qT"] = C(np.asarray(q[b], f32).T)
        m["kT"] = C(np.asarray(k[b], f32).T)
        m["vT"] = C(np.asarray(v[b], f32).T)
        m["cT"] = C(np.asarray(c[b], f32).T)
        in_maps.append(m)
    return flags, in_maps


def _run(in_maps, flags, trace=False):
    from concourse.bass_utils import run_bass_kernel_spmd
    key = tuple(sorted(flags.items()))
    if key not in _CACHE:
        _CACHE[key] = _build(flags)
    nc = _CACHE[key]
    res = run_bass_kernel_spmd(nc, in_maps, core_ids=list(range(B)),
                               trace=trace)
    out = np.stack([np.asarray(res.results[b]["Y"]) for b in range(B)])
    return out.astype(np.float32), res


def kernel(**inputs):
    flags, in_maps = _prep(**inputs)
    out, _ = _run(in_maps, flags, trace=False)
    return out


def kernel_timed(**inputs):
    flags, in_maps = _prep(**inputs)
    return _run(in_maps, flags, trace=True)
